# revision 17
# baseline (speedup 1.0000x reference)
"""Trainium2 Bass kernel for nn_CrossAttention_65051574665735.

Cross-attention block (MQA, shared K/V head) + parallel SwiGLU FF.
Data-parallel over B*N rows across 8 NeuronCores: core c handles batch c//4,
rows (c%4)*512. Context + weights replicated (weights pre-cast to bf16/fp8
with the layernorm scale g and the 1/sqrt(dh) attention scale folded in on
the host). No cross-core collectives; the host concatenates the 8 slices.

Schedule (single PE instruction stream, engines overlap via Tile semaphores):
  A: x layernorm + transpose -> xnT (bf16) and xnT8 (fp8 pairs)
  B: Q projection, fp8 DoubleRow (wq pre-scaled x256, fixed on PSUM copy)
  C: ctx layernorm/transpose + KV projection, FF1 units interleaved
  D: attention per head-pair (sim bf16, exp->fp8, AV fp8 DoubleRow),
     FF1 units interleaved to keep PE busy while ACT does exp
  E: Wo + FF2 accumulation, split by output column half for early drain
"""

import sys

if "/opt/trn_rl_repo" not in sys.path:
    sys.path.insert(0, "/opt/trn_rl_repo")

import numpy as np
import ml_dtypes

import concourse.bass as bass
import concourse.tile as tile
from concourse import mybir, bacc
from concourse.masks import make_identity

F32 = mybir.dt.float32
BF16 = mybir.dt.bfloat16
FP8 = mybir.dt.float8e4

B, N, J = 2, 2048, 2048
DIM, HEADS, DH = 1024, 16, 64
INNER = HEADS * DH
FF = 4 * DIM
EPS = 1e-5
N_CORES = 8
R = B * N // N_CORES  # 512 rows per core
KT = DIM // 128  # 8 contraction tiles over dim
KP = KT // 2  # 4 fp8 contraction pairs
RT = R // 128  # 4 row tiles
CT = J // 128  # 16 context row tiles
CP = CT // 2  # 8 context pairs for fp8 AV
FT = FF // 128  # 32 ff tiles
VO_W = 80  # vo pair inner stride (>=DH+1, 16B-aligned for DoubleRow)
QSC = 256.0  # host pre-scale on Wq for fp8


def _ln_tile(nc, pools, src_dram, t, bias_tile, out_dtype=BF16):
    """LN one 128-row tile of src_dram; returns normalized [128, DIM] tile."""
    ln_pool, stats_pool, eps_tile = pools
    x_t = ln_pool.tile([128, DIM], F32, tag="ln_x")
    nc.gpsimd.dma_start(x_t[:], src_dram[t * 128 : (t + 1) * 128, :])
    stats = stats_pool.tile([128, 2, nc.vector.BN_STATS_DIM], F32, tag="st")
    nc.vector.bn_stats(stats[:, 0, :], x_t[:, 0:512])
    nc.vector.bn_stats(stats[:, 1, :], x_t[:, 512:1024])
    mv = stats_pool.tile([128, nc.vector.BN_AGGR_DIM], F32, tag="mv")
    nc.vector.bn_aggr(mv[:], stats[:])
    rstd = stats_pool.tile([128, 1], F32, tag="rs")
    nc.scalar.activation(
        rstd[:], mv[:, 1:2], mybir.ActivationFunctionType.Sqrt, bias=eps_tile[:]
    )
    nc.vector.reciprocal(rstd[:], rstd[:])
    xn_t = ln_pool.tile([128, DIM], out_dtype, tag="ln_xn")
    nc.vector.tensor_scalar(
        out=xn_t[:],
        in0=x_t[:],
        scalar1=mv[:, 0:1],
        scalar2=rstd[:],
        op0=mybir.AluOpType.subtract,
        op1=mybir.AluOpType.mult,
    )
    if bias_tile is not None:
        nc.vector.tensor_add(xn_t[:], xn_t[:], bias_tile[:])
    return xn_t


def build_kernel(x_bias_nonzero: bool, c_bias_nonzero: bool):
    nc = bacc.Bacc(
        "TRN2", target_bir_lowering=False, debug=False, num_devices=N_CORES
    )
    d_x = nc.dram_tensor("x", [R, DIM], F32, kind="ExternalInput").ap()
    d_ctx = nc.dram_tensor("ctx", [J, DIM], F32, kind="ExternalInput").ap()
    d_wq8 = nc.dram_tensor("wq8", [DIM, INNER], FP8, kind="ExternalInput").ap()
    d_wkv = nc.dram_tensor("wkv", [DIM, 2 * DH], BF16, kind="ExternalInput").ap()
    d_wo = nc.dram_tensor("wo", [INNER, DIM], BF16, kind="ExternalInput").ap()
    d_wff1 = nc.dram_tensor("wff1", [DIM, 2 * FF], BF16, kind="ExternalInput").ap()
    d_wff2 = nc.dram_tensor("wff2", [FF, DIM], BF16, kind="ExternalInput").ap()
    d_xb = (
        nc.dram_tensor("xb", [1, DIM], F32, kind="ExternalInput").ap()
        if x_bias_nonzero
        else None
    )
    d_cb = (
        nc.dram_tensor("cb", [1, DIM], F32, kind="ExternalInput").ap()
        if c_bias_nonzero
        else None
    )
    d_out = nc.dram_tensor("out", [R, DIM], F32, kind="ExternalOutput").ap()

    with tile.TileContext(nc) as tc:
        with (
            tc.tile_pool(name="consts", bufs=1) as consts,
            tc.tile_pool(name="persist", bufs=1) as persist,
            tc.tile_pool(name="ln", bufs=3) as ln_pool,
            tc.tile_pool(name="stats", bufs=3) as stats_pool,
            tc.tile_pool(name="wo", bufs=1) as wo_pool,
        ):
            ident = consts.tile([128, 128], BF16)
            make_identity(nc, ident)
            eps_tile = consts.tile([128, 1], F32, tag="eps")
            nc.vector.memset(eps_tile[:], EPS)

            xb_tile = cb_tile = None
            if d_xb is not None:
                xb_tile = consts.tile([128, DIM], F32, tag="xb")
                nc.gpsimd.dma_start(
                    xb_tile[:],
                    bass.AP(
                        tensor=d_xb.tensor, offset=d_xb.offset,
                        ap=[[0, 128]] + d_xb.ap[1:],
                    ),
                )
            if d_cb is not None:
                cb_tile = consts.tile([128, DIM], F32, tag="cb")
                nc.gpsimd.dma_start(
                    cb_tile[:],
                    bass.AP(
                        tensor=d_cb.tensor, offset=d_cb.offset,
                        ap=[[0, 128]] + d_cb.ap[1:],
                    ),
                )

            xnT = [
                persist.tile([128, R], BF16, tag=f"xnT{k}", name=f"xnT{k}")
                for k in range(KT)
            ]
            xnT8 = [
                persist.tile([128, 2, R], FP8, tag=f"xnT8{p}", name=f"xnT8{p}")
                for p in range(KP)
            ]
            kT = persist.tile([128, J], BF16, tag="kT")
            vo8 = [
                persist.tile([128, 2, VO_W], FP8, tag=f"vo{p}", name=f"vo{p}")
                for p in range(CP)
            ]
            aoT = [
                persist.tile([128, R], BF16, tag=f"aoT{k}", name=f"aoT{k}")
                for k in range(KT)
            ]
            hT = [
                persist.tile([128, R], BF16, tag=f"hT{f}", name=f"hT{f}")
                for f in range(FT)
            ]
            qT = [
                persist.tile([128, R], BF16, tag=f"qT{h}", name=f"qT{h}")
                for h in range(HEADS // 2)
            ]
            ln_pools = (ln_pool, stats_pool, eps_tile)

            with (
                tc.tile_pool(name="wq8", bufs=1) as wq8_pool,
                tc.tile_pool(name="wff1", bufs=24) as wff1_pool,
                tc.tile_pool(name="sg", bufs=3) as sg_pool,
            ):
                # ---- weight DMAs (sync queue, in consumption order) ----
                wq8_sb = [
                    wq8_pool.tile([128, 2, INNER], FP8, tag=f"wq{p}", name=f"wq{p}")
                    for p in range(KP)
                ]
                for p in range(KP):
                    for i in range(2):
                        nc.sync.dma_start(
                            wq8_sb[p][:, i, :],
                            d_wq8[(2 * p + i) * 128 : (2 * p + i + 1) * 128, :],
                        )
                wo_sb = [
                    wo_pool.tile([128, DIM], BF16, tag=f"wo{k}", name=f"wo{k}")
                    for k in range(KT)
                ]

                # ---- FF1 unit generator (paced into phases C and D) ----
                w1_tiles = {}

                def ff1_steps(fi):
                    g = fi // 8
                    fl = fi % 8
                    if fl == 0:
                        w1a = []
                        w1g = []
                        for k in range(KT):
                            ta = wff1_pool.tile([128, 1024], BF16, tag="w1")
                            nc.sync.dma_start(
                                ta[:],
                                d_wff1[
                                    k * 128 : (k + 1) * 128,
                                    g * 1024 : (g + 1) * 1024,
                                ],
                            )
                            w1a.append(ta)
                            tg = wff1_pool.tile([128, 1024], BF16, tag="w1")
                            nc.sync.dma_start(
                                tg[:],
                                d_wff1[
                                    k * 128 : (k + 1) * 128,
                                    FF + g * 1024 : FF + (g + 1) * 1024,
                                ],
                            )
                            w1g.append(tg)
                        w1_tiles[g] = (w1a, w1g)
                    w1a, w1g = w1_tiles[g]
                    a_ps = psum_f.tile([128, R], F32, tag="ffa")
                    g_ps = psum_f.tile([128, R], F32, tag="ffg")

                    def mk_chain(ps, w1, k0):
                        def emit():
                            for k in range(k0, k0 + 4):
                                nc.tensor.matmul(
                                    ps[:],
                                    w1[k][:, fl * 128 : (fl + 1) * 128],
                                    xnT[k][:],
                                    start=(k == 0),
                                    stop=(k == KT - 1),
                                )
                        return emit

                    def finish():
                        for k in range(4, 8):
                            nc.tensor.matmul(
                                g_ps[:],
                                w1g[k][:, fl * 128 : (fl + 1) * 128],
                                xnT[k][:],
                                start=False,
                                stop=(k == KT - 1),
                            )
                        sg = sg_pool.tile([128, R], F32, tag="sg")
                        nc.scalar.activation(
                            sg[:], g_ps[:], mybir.ActivationFunctionType.Silu
                        )
                        nc.vector.tensor_mul(hT[fi][:], a_ps[:], sg[:])

                    return [
                        mk_chain(a_ps, w1a, 0),
                        mk_chain(a_ps, w1a, 4),
                        mk_chain(g_ps, w1g, 0),
                        finish,
                    ]

                ff_queue = []
                ff_next = [0]

                def ff_step(n=1):
                    for _ in range(n):
                        if not ff_queue and ff_next[0] < FT:
                            ff_queue.extend(ff1_steps(ff_next[0]))
                            ff_next[0] += 1
                        if ff_queue:
                            ff_queue.pop(0)()

                with (
                    tc.tile_pool(name="cnT", bufs=1) as cnT_pool,
                    tc.tile_pool(name="psA", bufs=2, space="PSUM") as psum_tr,
                    tc.tile_pool(name="psF", bufs=1, space="PSUM") as psum_f,
                    tc.tile_pool(name="psKV", bufs=2, space="PSUM") as psum_kv,
                    tc.tile_pool(name="wkv", bufs=1) as wkv_pool,
                    tc.tile_pool(name="vstage", bufs=2) as vstage,
                    tc.tile_pool(name="psQ", bufs=2, space="PSUM") as psum_q,
                ):
                    # ---- Phase A: x layernorm + transposes ----
                    for t in range(RT):
                        xn_t = _ln_tile(nc, ln_pools, d_x, t, xb_tile)
                        for k in range(KT):
                            ps = psum_tr.tile([128, 128], BF16, tag="tr")
                            nc.tensor.transpose(
                                ps[:], xn_t[:, k * 128 : (k + 1) * 128], ident[:]
                            )
                            nc.vector.tensor_copy(
                                xnT[k][:, t * 128 : (t + 1) * 128], ps[:]
                            )
                            nc.vector.tensor_copy(
                                xnT8[k // 2][:, k % 2, t * 128 : (t + 1) * 128],
                                ps[:],
                            )

                    # ---- Phase B: Q projection (fp8 DoubleRow) ----
                    for hp in range(HEADS // 2):
                        q_ps = psum_q.tile([128, R], F32, tag="q")
                        for p in range(KP):
                            nc.tensor.matmul(
                                q_ps[:],
                                wq8_sb[p][:, :, hp * 128 : (hp + 1) * 128],
                                xnT8[p][:],
                                start=(p == 0),
                                stop=(p == KP - 1),
                                perf_mode=mybir.MatmulPerfMode.DoubleRow,
                            )
                        nc.scalar.activation(
                            qT[hp][:],
                            q_ps[:],
                            mybir.ActivationFunctionType.Copy,
                            scale=1.0 / QSC,
                        )

                    wkv_sb = [
                        wkv_pool.tile(
                            [128, 2 * DH], BF16, tag=f"wkv{k}", name=f"wkv{k}"
                        )
                        for k in range(KT)
                    ]
                    for k in range(KT):
                        nc.sync.dma_start(
                            wkv_sb[k][:], d_wkv[k * 128 : (k + 1) * 128, :]
                        )
                    for p in range(CP):
                        nc.vector.memset(vo8[p][:, 0, DH : DH + 1], 1.0)
                        nc.vector.memset(vo8[p][:, 1, DH : DH + 1], 1.0)
                        nc.vector.memset(vo8[p][:, 0, DH + 1 : VO_W], 0.0)
                        nc.vector.memset(vo8[p][:, 1, DH + 1 : VO_W], 0.0)

                    # ---- Phase C: ctx layernorm/transpose + KV projection ----
                    cnT = [
                        cnT_pool.tile(
                            [128, J], BF16, tag=f"cnT{k}", name=f"cnT{k}"
                        )
                        for k in range(KT)
                    ]
                    for c in range(J // 512):
                        for t4 in range(4):
                            t = c * 4 + t4
                            cn_t = _ln_tile(nc, ln_pools, d_ctx, t, cb_tile)
                            for k in range(KT):
                                ps = psum_tr.tile([128, 128], BF16, tag="tr")
                                nc.tensor.transpose(
                                    ps[:], cn_t[:, k * 128 : (k + 1) * 128], ident[:]
                                )
                                nc.vector.tensor_copy(
                                    cnT[k][:, t * 128 : (t + 1) * 128], ps[:]
                                )
                        kv_ps = psum_kv.tile([128, 512], F32, tag="kv")
                        for k in range(KT):
                            nc.tensor.matmul(
                                kv_ps[:],
                                wkv_sb[k][:],
                                cnT[k][:, c * 512 : (c + 1) * 512],
                                start=(k == 0),
                                stop=(k == KT - 1),
                            )
                        nc.scalar.copy(
                            kT[0:DH, c * 512 : (c + 1) * 512], kv_ps[0:DH, :]
                        )
                        nc.gpsimd.dma_start(
                            kT[DH:128, c * 512 : (c + 1) * 512],
                            kT[0:DH, c * 512 : (c + 1) * 512],
                        )
                        vT_sb = vstage.tile([128, 512], BF16, tag="vT")
                        nc.vector.tensor_copy(vT_sb[DH:128, :], kv_ps[DH:128, :])
                        for j4 in range(4):
                            jc = c * 4 + j4
                            vps = psum_tr.tile([128, DH], BF16, tag="tr")
                            nc.tensor.transpose(
                                vps[:],
                                vT_sb[DH:128, j4 * 128 : (j4 + 1) * 128],
                                ident[DH:128, DH:128],
                            )
                            nc.vector.tensor_copy(
                                vo8[jc // 2][:, jc % 2, 0:DH], vps[:]
                            )
                        if c >= 1:
                            ff_step(4)

                # ---- Phase D: attention, FF1 interleaved ----
                with (
                    tc.tile_pool(name="apair", bufs=10) as apair_pool,
                    tc.tile_pool(name="smx", bufs=4) as smx_pool,
                    tc.tile_pool(name="psS", bufs=2, space="PSUM") as psum_s,
                    tc.tile_pool(name="psAV", bufs=2, space="PSUM") as psum_av,
                    tc.tile_pool(name="psF", bufs=2, space="PSUM") as psum_f2,
                ):
                    psum_f = psum_f2
                    for hp in range(HEADS // 2):
                        av_ps = [None, None]
                        for h2 in range(2):
                            qh = qT[hp][h2 * 64 : (h2 + 1) * 64, :]
                            a_pairs = []
                            for p in range(CP):
                                ap8 = apair_pool.tile([128, 2, R], FP8, tag="ap8")
                                for half in range(2):
                                    jc = 2 * p + half
                                    s_ps = psum_s.tile([128, R], F32, tag="sim")
                                    nc.tensor.matmul(
                                        s_ps[:],
                                        kT[
                                            h2 * DH : (h2 + 1) * DH,
                                            jc * 128 : (jc + 1) * 128,
                                        ],
                                        qh,
                                        start=True,
                                        stop=True,
                                    )
                                    nc.scalar.activation(
                                        ap8[:, half, :],
                                        s_ps[:],
                                        mybir.ActivationFunctionType.Exp,
                                    )
                                a_pairs.append(ap8)
                                ff_step(1)
                            av = psum_av.tile([VO_W, R], F32, tag="av")
                            av_ps[h2] = av
                            for p in range(CP):
                                nc.tensor.matmul(
                                    av[:],
                                    vo8[p][:],
                                    a_pairs[p][:],
                                    start=(p == 0),
                                    stop=(p == CP - 1),
                                    perf_mode=mybir.MatmulPerfMode.DoubleRow,
                                )
                                if p in (2, 5):
                                    ff_step(1)
                        for h2 in range(2):
                            av = av_ps[h2]
                            rec = smx_pool.tile([DH + 1, R], F32, tag="rec")
                            nc.vector.reciprocal(
                                rec[DH : DH + 1, :], av[DH : DH + 1, :]
                            )
                            rec0 = smx_pool.tile([1, R], F32, tag="rec0")
                            nc.gpsimd.dma_start(rec0[:], rec[DH : DH + 1, :])
                            rbc = smx_pool.tile([DH, R], F32, tag="rbc")
                            nc.gpsimd.partition_broadcast(rbc[:], rec0[:])
                            if h2 == 0:
                                nc.vector.tensor_mul(
                                    aoT[hp][0:DH, :], av_ps[0][0:DH, :], rbc[:]
                                )
                            else:
                                tmp = smx_pool.tile([DH, R], BF16, tag="aotmp")
                                nc.vector.tensor_mul(
                                    tmp[:], av_ps[1][0:DH, :], rbc[:]
                                )
                                nc.gpsimd.dma_start(aoT[hp][DH:128, :], tmp[:])
                        ff_step(1)
                        if hp == 5:
                            for k in range(KT):
                                nc.sync.dma_start(
                                    wo_sb[k][:], d_wo[k * 128 : (k + 1) * 128, :]
                                )
                    while ff_queue or ff_next[0] < FT:
                        ff_step(1)

            # ---- Phase E: Wo + FF2, split by output column half ----
            with (
                tc.tile_pool(name="wff2", bufs=8) as wff2_pool,
                tc.tile_pool(name="ostage", bufs=4) as ostage,
                tc.tile_pool(name="psO", bufs=1, space="PSUM") as psum_o,
            ):
                for ch in range(2):
                    o_ps = [
                        psum_o.tile([128, 512], F32, tag=f"o{ch}{rs}", name=f"o{ch}{rs}")
                        for rs in range(RT)
                    ]
                    for k in range(KT):
                        for rs in range(RT):
                            nc.tensor.matmul(
                                o_ps[rs][:],
                                aoT[k][:, rs * 128 : (rs + 1) * 128],
                                wo_sb[k][:, ch * 512 : (ch + 1) * 512],
                                start=(k == 0),
                                stop=False,
                            )
                    for fi in range(FT):
                        w2 = wff2_pool.tile([128, 512], BF16, tag="w2")
                        nc.sync.dma_start(
                            w2[:],
                            d_wff2[
                                fi * 128 : (fi + 1) * 128,
                                ch * 512 : (ch + 1) * 512,
                            ],
                        )
                        for rs in range(RT):
                            nc.tensor.matmul(
                                o_ps[rs][:],
                                hT[fi][:, rs * 128 : (rs + 1) * 128],
                                w2[:],
                                start=False,
                                stop=(fi == FT - 1),
                            )
                    for rs in range(RT):
                        o_sb = ostage.tile([128, 512], F32, tag="ost")
                        nc.scalar.copy(o_sb[:], o_ps[rs][:])
                        nc.gpsimd.dma_start(
                            d_out[
                                rs * 128 : (rs + 1) * 128,
                                ch * 512 : (ch + 1) * 512,
                            ],
                            o_sb[:],
                        )

    nc.compile()
    return nc


_NC_CACHE = {}


def _get_nc(x_bias_nonzero, c_bias_nonzero):
    key = (x_bias_nonzero, c_bias_nonzero)
    if key not in _NC_CACHE:
        _NC_CACHE[key] = build_kernel(*key)
    return _NC_CACHE[key]


def make_in_maps(x, context, norm_g, norm_b, cnorm_g, cnorm_b, Wq, Wkv, Wo, Wff1, Wff2):
    x = np.asarray(x, np.float32)
    context = np.asarray(context, np.float32)
    norm_g = np.asarray(norm_g, np.float32)
    norm_b = np.asarray(norm_b, np.float32)
    cnorm_g = np.asarray(cnorm_g, np.float32)
    cnorm_b = np.asarray(cnorm_b, np.float32)
    scale = DH ** -0.5
    bf = ml_dtypes.bfloat16
    f8 = ml_dtypes.float8_e4m3
    wq8 = np.ascontiguousarray(
        np.clip(
            norm_g[:, None] * np.asarray(Wq, np.float32) * scale * QSC, -240, 240
        )
    ).astype(f8)
    wkv = np.ascontiguousarray(cnorm_g[:, None] * np.asarray(Wkv, np.float32)).astype(bf)
    wo = np.ascontiguousarray(np.asarray(Wo, np.float32)).astype(bf)
    wff1 = np.ascontiguousarray(norm_g[:, None] * np.asarray(Wff1, np.float32)).astype(bf)
    wff2 = np.ascontiguousarray(np.asarray(Wff2, np.float32)).astype(bf)
    x_bias = bool(np.any(norm_b != 0.0))
    c_bias = bool(np.any(cnorm_b != 0.0))
    in_maps = []
    for c in range(N_CORES):
        b = c // (N_CORES // B)
        r0 = (c % (N_CORES // B)) * R
        m = {
            "x": np.ascontiguousarray(x[b, r0 : r0 + R, :]),
            "ctx": np.ascontiguousarray(context[b]),
            "wq8": wq8,
            "wkv": wkv,
            "wo": wo,
            "wff1": wff1,
            "wff2": wff2,
        }
        if x_bias:
            m["xb"] = norm_b.reshape(1, DIM).copy()
        if c_bias:
            m["cb"] = cnorm_b.reshape(1, DIM).copy()
        in_maps.append(m)
    return in_maps, x_bias, c_bias


def gather_output(results):
    out = np.empty((B, N, DIM), np.float32)
    for c in range(N_CORES):
        b = c // (N_CORES // B)
        r0 = (c % (N_CORES // B)) * R
        out[b, r0 : r0 + R, :] = results[c]["out"]
    return out


def kernel(**inputs):
    from concourse.bass_utils import run_bass_kernel_spmd

    in_maps, x_bias, c_bias = make_in_maps(**inputs)
    nc = _get_nc(x_bias, c_bias)
    res = run_bass_kernel_spmd(nc, in_maps, list(range(N_CORES)))
    return gather_output(res.results)


# revision 21
# speedup vs baseline: 1.1178x; 1.1178x over previous
"""Trainium2 Bass kernel for nn_CrossAttention_65051574665735.

Cross-attention block (MQA, shared K/V head) + parallel SwiGLU FF.
Data-parallel over B*N rows across 8 NeuronCores: core c handles batch c//4,
rows (c%4)*512. Context + weights replicated (weights pre-cast to bf16/fp8
with the layernorm scale g and the 1/sqrt(dh) attention scale folded in on
the host). No cross-core collectives; the host concatenates the 8 slices.

Schedule notes:
- All layernorm transposes go through the DMA crossbar (dma_start_transpose),
  keeping PE for matmuls and DVE for element-wise work.
- The scalar (ACT) engine runs a single activation function per phase to
  avoid act-table reloads: Sqrt during layernorms, Exp during attention,
  Silu during the FF down-projection.
- FF1 up-proj matmuls are emitted in small "steps" interleaved into the ctx
  and attention phases to fill PE gaps; the SwiGLU nonlinearity is deferred
  to phase E (raw a/gate stored in SBUF as bf16).
- Q projection uses fp8 DoubleRow (weights pre-scaled x256 on the host,
  un-scaled on the PSUM->SBUF copy).
- Phase E accumulates Wo + FF2 into PSUM split by output column half so the
  first half drains (copy + store) while the second half computes.
"""

import sys

if "/opt/trn_rl_repo" not in sys.path:
    sys.path.insert(0, "/opt/trn_rl_repo")

import numpy as np
import ml_dtypes

import concourse.bass as bass
import concourse.tile as tile
from concourse import mybir, bacc
from concourse.masks import make_identity

F32 = mybir.dt.float32
BF16 = mybir.dt.bfloat16
FP8 = mybir.dt.float8e4

B, N, J = 2, 2048, 2048
DIM, HEADS, DH = 1024, 16, 64
INNER = HEADS * DH
FF = 4 * DIM
EPS = 1e-5
N_CORES = 8
R = B * N // N_CORES  # 512 rows per core
KT = DIM // 128  # 8 contraction tiles over dim
KP = KT // 2  # 4 fp8 contraction pairs
RT = R // 128  # 4 row tiles
CT = J // 128  # 16 context row tiles
FT = FF // 128  # 32 ff tiles
QSC = 256.0  # host pre-scale on Wq for fp8


def _ln_tile(nc, pools, src_dram, t, bias_tile):
    """LN one 128-row tile of src_dram; returns normalized [128, DIM] bf16."""
    ln_pool, stats_pool, eps_tile = pools
    x_t = ln_pool.tile([128, DIM], F32, tag="ln_x")
    nc.gpsimd.dma_start(x_t[:], src_dram[t * 128 : (t + 1) * 128, :])
    stats = stats_pool.tile([128, 2, nc.vector.BN_STATS_DIM], F32, tag="st")
    nc.vector.bn_stats(stats[:, 0, :], x_t[:, 0:512])
    nc.vector.bn_stats(stats[:, 1, :], x_t[:, 512:1024])
    mv = stats_pool.tile([128, nc.vector.BN_AGGR_DIM], F32, tag="mv")
    nc.vector.bn_aggr(mv[:], stats[:])
    rstd = stats_pool.tile([128, 1], F32, tag="rs")
    nc.scalar.activation(
        rstd[:], mv[:, 1:2], mybir.ActivationFunctionType.Sqrt, bias=eps_tile[:]
    )
    nc.vector.reciprocal(rstd[:], rstd[:])
    xn_t = ln_pool.tile([128, DIM], BF16, tag="ln_xn")
    nc.vector.tensor_scalar(
        out=xn_t[:],
        in0=x_t[:],
        scalar1=mv[:, 0:1],
        scalar2=rstd[:],
        op0=mybir.AluOpType.subtract,
        op1=mybir.AluOpType.mult,
    )
    if bias_tile is not None:
        nc.vector.tensor_add(xn_t[:], xn_t[:], bias_tile[:])
    return xn_t


def build_kernel(x_bias_nonzero: bool, c_bias_nonzero: bool):
    nc = bacc.Bacc(
        "TRN2", target_bir_lowering=False, debug=False, num_devices=N_CORES
    )
    d_x = nc.dram_tensor("x", [R, DIM], F32, kind="ExternalInput").ap()
    d_ctx = nc.dram_tensor("ctx", [J, DIM], F32, kind="ExternalInput").ap()
    d_wq8 = nc.dram_tensor("wq8", [DIM, INNER], FP8, kind="ExternalInput").ap()
    d_wkv = nc.dram_tensor("wkv", [DIM, 2 * DH], BF16, kind="ExternalInput").ap()
    d_wo = nc.dram_tensor("wo", [INNER, DIM], BF16, kind="ExternalInput").ap()
    d_wff1 = nc.dram_tensor("wff1", [DIM, 2 * FF], BF16, kind="ExternalInput").ap()
    d_wff2 = nc.dram_tensor("wff2", [FF, DIM], BF16, kind="ExternalInput").ap()
    d_xb = (
        nc.dram_tensor("xb", [1, DIM], F32, kind="ExternalInput").ap()
        if x_bias_nonzero
        else None
    )
    d_cb = (
        nc.dram_tensor("cb", [1, DIM], F32, kind="ExternalInput").ap()
        if c_bias_nonzero
        else None
    )
    d_out = nc.dram_tensor("out", [R, DIM], F32, kind="ExternalOutput").ap()

    with tile.TileContext(nc) as tc:
        with (
            tc.tile_pool(name="consts", bufs=1) as consts,
            tc.tile_pool(name="persist", bufs=1) as persist,
            tc.tile_pool(name="wo", bufs=1) as wo_pool,
        ):
            ident = consts.tile([128, 128], BF16)
            make_identity(nc, ident)
            eps_tile = consts.tile([128, 1], F32, tag="eps")
            nc.vector.memset(eps_tile[:], EPS)

            xb_tile = cb_tile = None
            if d_xb is not None:
                xb_tile = consts.tile([128, DIM], F32, tag="xb")
                nc.gpsimd.dma_start(
                    xb_tile[:],
                    bass.AP(
                        tensor=d_xb.tensor, offset=d_xb.offset,
                        ap=[[0, 128]] + d_xb.ap[1:],
                    ),
                )
            if d_cb is not None:
                cb_tile = consts.tile([128, DIM], F32, tag="cb")
                nc.gpsimd.dma_start(
                    cb_tile[:],
                    bass.AP(
                        tensor=d_cb.tensor, offset=d_cb.offset,
                        ap=[[0, 128]] + d_cb.ap[1:],
                    ),
                )

            xnT = persist.tile([128, KT, R], BF16, tag="xnT")
            xnT8 = [
                persist.tile([128, 2, R], FP8, tag=f"xnT8{p}", name=f"xnT8{p}")
                for p in range(KP)
            ]
            kT = persist.tile([128, J], BF16, tag="kT")
            vo = [
                persist.tile([128, DH + 1], BF16, tag=f"vo{j}", name=f"vo{j}")
                for j in range(CT)
            ]
            aoT = [
                persist.tile([128, R], BF16, tag=f"aoT{k}", name=f"aoT{k}")
                for k in range(KT)
            ]
            qT = [
                persist.tile([128, R], BF16, tag=f"qT{h}", name=f"qT{h}")
                for h in range(HEADS // 2)
            ]
            aT = [
                persist.tile([128, R], BF16, tag=f"aT{f}", name=f"aT{f}")
                for f in range(FT)
            ]
            gT = [
                persist.tile([128, R], BF16, tag=f"gT{f}", name=f"gT{f}")
                for f in range(FT)
            ]
            wo_sb = [
                wo_pool.tile([128, DIM], BF16, tag=f"wo{k}", name=f"wo{k}")
                for k in range(KT)
            ]

            with tc.tile_pool(name="wff1", bufs=16) as wff1_pool:
                # ---- FF1 unit generator (paced into phases C and D) ----
                w1_tiles = {}

                def ff1_steps(fi):
                    g = fi // 8
                    fl = fi % 8
                    if fl == 0:
                        w1a = []
                        w1g = []
                        for k in range(KT):
                            ta = wff1_pool.tile([128, 1024], BF16, tag="w1")
                            nc.sync.dma_start(
                                ta[:],
                                d_wff1[
                                    k * 128 : (k + 1) * 128,
                                    g * 1024 : (g + 1) * 1024,
                                ],
                            )
                            w1a.append(ta)
                            tg = wff1_pool.tile([128, 1024], BF16, tag="w1")
                            nc.sync.dma_start(
                                tg[:],
                                d_wff1[
                                    k * 128 : (k + 1) * 128,
                                    FF + g * 1024 : FF + (g + 1) * 1024,
                                ],
                            )
                            w1g.append(tg)
                        w1_tiles[g] = (w1a, w1g)
                    w1a, w1g = w1_tiles[g]
                    a_ps = psum_f.tile([128, R], F32, tag="ffa")
                    g_ps = psum_f.tile([128, R], F32, tag="ffg")

                    def mk_chain(ps, w1, k0):
                        def emit():
                            for k in range(k0, k0 + 4):
                                nc.tensor.matmul(
                                    ps[:],
                                    w1[k][:, fl * 128 : (fl + 1) * 128],
                                    xnT[:, k, :],
                                    start=(k == 0),
                                    stop=(k == KT - 1),
                                )
                        return emit

                    def finish():
                        for k in range(4, 8):
                            nc.tensor.matmul(
                                g_ps[:],
                                w1g[k][:, fl * 128 : (fl + 1) * 128],
                                xnT[:, k, :],
                                start=False,
                                stop=(k == KT - 1),
                            )
                        nc.vector.tensor_copy(aT[fi][:], a_ps[:])
                        nc.vector.tensor_copy(gT[fi][:], g_ps[:])

                    return [
                        mk_chain(a_ps, w1a, 0),
                        mk_chain(a_ps, w1a, 4),
                        mk_chain(g_ps, w1g, 0),
                        finish,
                    ]

                ff_queue = []
                ff_next = [0]

                def ff_step(n=1):
                    for _ in range(n):
                        if not ff_queue and ff_next[0] < FT:
                            ff_queue.extend(ff1_steps(ff_next[0]))
                            ff_next[0] += 1
                        if ff_queue:
                            ff_queue.pop(0)()

                with (
                    tc.tile_pool(name="ln", bufs=2) as ln_pool,
                    tc.tile_pool(name="stats", bufs=3) as stats_pool,
                    tc.tile_pool(name="wq8", bufs=1) as wq8_pool,
                    tc.tile_pool(name="cnT", bufs=1) as cnT_pool,
                    tc.tile_pool(name="wkv", bufs=1) as wkv_pool,
                    tc.tile_pool(name="vstage", bufs=2) as vstage,
                    tc.tile_pool(name="psA", bufs=2, space="PSUM") as psum_tr,
                    tc.tile_pool(name="psF", bufs=1, space="PSUM") as psum_f,
                    tc.tile_pool(name="psKV", bufs=2, space="PSUM") as psum_kv,
                    tc.tile_pool(name="psQ", bufs=2, space="PSUM") as psum_q,
                ):
                    ln_pools = (ln_pool, stats_pool, eps_tile)
                    wq8_sb = [
                        wq8_pool.tile(
                            [128, 2, INNER], FP8, tag=f"wq{p}", name=f"wq{p}"
                        )
                        for p in range(KP)
                    ]
                    for p in range(KP):
                        for i in range(2):
                            nc.sync.dma_start(
                                wq8_sb[p][:, i, :],
                                d_wq8[(2 * p + i) * 128 : (2 * p + i + 1) * 128, :],
                            )
                    wkv_sb = [
                        wkv_pool.tile(
                            [128, 2 * DH], BF16, tag=f"wkv{k}", name=f"wkv{k}"
                        )
                        for k in range(KT)
                    ]
                    for k in range(KT):
                        nc.sync.dma_start(
                            wkv_sb[k][:], d_wkv[k * 128 : (k + 1) * 128, :]
                        )

                    # ---- Phase A: x layernorm + DMA transposes ----
                    for t in range(RT):
                        xn_t = _ln_tile(nc, ln_pools, d_x, t, xb_tile)
                        nc.scalar.dma_start_transpose(
                            xnT[:, :, t * 128 : (t + 1) * 128], xn_t[:]
                        )
                    for p in range(KP):
                        for i in range(2):
                            nc.vector.tensor_copy(
                                xnT8[p][:, i, :], xnT[:, 2 * p + i, :]
                            )

                    # ---- Phase B: Q projection (fp8 DoubleRow) ----
                    for hp in range(HEADS // 2):
                        q_ps = psum_q.tile([128, R], F32, tag="q")
                        for p in range(KP):
                            nc.tensor.matmul(
                                q_ps[:],
                                wq8_sb[p][:, :, hp * 128 : (hp + 1) * 128],
                                xnT8[p][:],
                                start=(p == 0),
                                stop=(p == KP - 1),
                                perf_mode=mybir.MatmulPerfMode.DoubleRow,
                            )
                        nc.vector.tensor_scalar(
                            out=qT[hp][:],
                            in0=q_ps[:],
                            scalar1=1.0 / QSC,
                            scalar2=None,
                            op0=mybir.AluOpType.mult,
                        )

                    # ---- Phase C: ctx layernorm + KV projection ----
                    cnT = cnT_pool.tile([128, KT, J], BF16, tag="cnT")
                    for c in range(J // 512):
                        for t4 in range(4):
                            t = c * 4 + t4
                            cn_t = _ln_tile(nc, ln_pools, d_ctx, t, cb_tile)
                            nc.scalar.dma_start_transpose(
                                cnT[:, :, t * 128 : (t + 1) * 128], cn_t[:]
                            )
                        kv_ps = psum_kv.tile([128, 512], F32, tag="kv")
                        for k in range(KT):
                            nc.tensor.matmul(
                                kv_ps[:],
                                wkv_sb[k][:],
                                cnT[:, k, c * 512 : (c + 1) * 512],
                                start=(k == 0),
                                stop=(k == KT - 1),
                            )
                        nc.vector.tensor_copy(
                            kT[0:DH, c * 512 : (c + 1) * 512], kv_ps[0:DH, :]
                        )
                        nc.gpsimd.dma_start(
                            kT[DH:128, c * 512 : (c + 1) * 512],
                            kT[0:DH, c * 512 : (c + 1) * 512],
                        )
                        vT_sb = vstage.tile([128, 512], BF16, tag="vT")
                        nc.vector.tensor_copy(vT_sb[DH:128, :], kv_ps[DH:128, :])
                        for j4 in range(4):
                            jc = c * 4 + j4
                            vps = psum_tr.tile([128, DH], BF16, tag="tr")
                            nc.tensor.transpose(
                                vps[:],
                                vT_sb[DH:128, j4 * 128 : (j4 + 1) * 128],
                                ident[DH:128, DH:128],
                            )
                            nc.vector.tensor_copy(vo[jc][:, 0:DH], vps[:])
                            nc.vector.memset(vo[jc][:, DH : DH + 1], 1.0)
                        if c >= 1:
                            ff_step(4)

                # ---- Phase D: attention, FF1 interleaved ----
                with (
                    tc.tile_pool(name="attn", bufs=9) as attn_pool,
                    tc.tile_pool(name="smx", bufs=4) as smx_pool,
                    tc.tile_pool(name="psS", bufs=2, space="PSUM") as psum_s,
                    tc.tile_pool(name="psAV", bufs=2, space="PSUM") as psum_av,
                    tc.tile_pool(name="psF2", bufs=1, space="PSUM") as psum_f2,
                ):
                    psum_f = psum_f2
                    for hp in range(HEADS // 2):
                        av_ps = [None, None]
                        for h2 in range(2):
                            qh = qT[hp][h2 * 64 : (h2 + 1) * 64, :]
                            a_sbs = []
                            for p in range(CT // 2):
                                s_ps = psum_s.tile([128, 2 * R], F32, tag="sim")
                                for half in range(2):
                                    jc = 2 * p + half
                                    nc.tensor.matmul(
                                        s_ps[:, half * R : (half + 1) * R],
                                        kT[
                                            h2 * DH : (h2 + 1) * DH,
                                            jc * 128 : (jc + 1) * 128,
                                        ],
                                        qh,
                                        start=True,
                                        stop=True,
                                    )
                                a_sb = attn_pool.tile([128, 2 * R], BF16, tag="at")
                                nc.scalar.activation(
                                    a_sb[:],
                                    s_ps[:],
                                    mybir.ActivationFunctionType.Exp,
                                )
                                a_sbs.append(a_sb)
                                ff_step(1)
                            av = psum_av.tile([DH + 1, R], F32, tag="av")
                            av_ps[h2] = av
                            for jc in range(CT):
                                nc.tensor.matmul(
                                    av[:],
                                    vo[jc][:],
                                    a_sbs[jc // 2][
                                        :, (jc % 2) * R : (jc % 2 + 1) * R
                                    ],
                                    start=(jc == 0),
                                    stop=(jc == CT - 1),
                                )
                                if jc in (5, 11):
                                    ff_step(1)
                        for h2 in range(2):
                            av = av_ps[h2]
                            rec = smx_pool.tile([DH + 1, R], F32, tag="rec")
                            nc.vector.reciprocal(
                                rec[DH : DH + 1, :], av[DH : DH + 1, :]
                            )
                            rec0 = smx_pool.tile([1, R], F32, tag="rec0")
                            nc.gpsimd.dma_start(rec0[:], rec[DH : DH + 1, :])
                            rbc = smx_pool.tile([DH, R], F32, tag="rbc")
                            nc.gpsimd.partition_broadcast(rbc[:], rec0[:])
                            if h2 == 0:
                                nc.vector.tensor_mul(
                                    aoT[hp][0:DH, :], av[0:DH, :], rbc[:]
                                )
                            else:
                                tmp = smx_pool.tile([DH, R], BF16, tag="aotmp")
                                nc.vector.tensor_mul(tmp[:], av[0:DH, :], rbc[:])
                                nc.gpsimd.dma_start(aoT[hp][DH:128, :], tmp[:])
                        ff_step(1)
                        if hp == 5:
                            for k in range(KT):
                                nc.sync.dma_start(
                                    wo_sb[k][:], d_wo[k * 128 : (k + 1) * 128, :]
                                )
                    while ff_queue or ff_next[0] < FT:
                        ff_step(1)

            # ---- Phase E: SwiGLU + Wo + FF2, split by output column half ----
            with (
                tc.tile_pool(name="wff2", bufs=8) as wff2_pool,
                tc.tile_pool(name="hpool", bufs=1) as hpool,
                tc.tile_pool(name="hstage", bufs=3) as hstage,
                tc.tile_pool(name="ostage", bufs=4) as ostage,
                tc.tile_pool(name="psO", bufs=1, space="PSUM") as psum_o,
            ):
                hT = {}
                for ch in range(2):
                    o_ps = [
                        psum_o.tile(
                            [128, 512], F32, tag=f"o{ch}{rs}", name=f"o{ch}{rs}"
                        )
                        for rs in range(RT)
                    ]
                    for k in range(KT):
                        for rs in range(RT):
                            nc.tensor.matmul(
                                o_ps[rs][:],
                                aoT[k][:, rs * 128 : (rs + 1) * 128],
                                wo_sb[k][:, ch * 512 : (ch + 1) * 512],
                                start=(k == 0),
                                stop=False,
                            )
                    for fi in range(FT):
                        w2 = wff2_pool.tile([128, 512], BF16, tag="w2")
                        nc.sync.dma_start(
                            w2[:],
                            d_wff2[
                                fi * 128 : (fi + 1) * 128,
                                ch * 512 : (ch + 1) * 512,
                            ],
                        )
                        if ch == 0:
                            sil = hstage.tile([128, R], F32, tag="sil")
                            nc.scalar.activation(
                                sil[:],
                                gT[fi][:],
                                mybir.ActivationFunctionType.Silu,
                            )
                            h = hT[fi] = hpool.tile(
                                [128, R], BF16, tag=f"h{fi}", name=f"h{fi}"
                            )
                            nc.vector.tensor_mul(h[:], aT[fi][:], sil[:])
                        for rs in range(RT):
                            nc.tensor.matmul(
                                o_ps[rs][:],
                                hT[fi][:, rs * 128 : (rs + 1) * 128],
                                w2[:],
                                start=False,
                                stop=(fi == FT - 1),
                            )
                    for rs in range(RT):
                        o_sb = ostage.tile([128, 512], F32, tag="ost")
                        nc.vector.tensor_copy(o_sb[:], o_ps[rs][:])
                        nc.gpsimd.dma_start(
                            d_out[
                                rs * 128 : (rs + 1) * 128,
                                ch * 512 : (ch + 1) * 512,
                            ],
                            o_sb[:],
                        )

    nc.compile()
    return nc


_NC_CACHE = {}


def _get_nc(x_bias_nonzero, c_bias_nonzero):
    key = (x_bias_nonzero, c_bias_nonzero)
    if key not in _NC_CACHE:
        _NC_CACHE[key] = build_kernel(*key)
    return _NC_CACHE[key]


def make_in_maps(x, context, norm_g, norm_b, cnorm_g, cnorm_b, Wq, Wkv, Wo, Wff1, Wff2):
    x = np.asarray(x, np.float32)
    context = np.asarray(context, np.float32)
    norm_g = np.asarray(norm_g, np.float32)
    norm_b = np.asarray(norm_b, np.float32)
    cnorm_g = np.asarray(cnorm_g, np.float32)
    cnorm_b = np.asarray(cnorm_b, np.float32)
    scale = DH ** -0.5
    bf = ml_dtypes.bfloat16
    f8 = ml_dtypes.float8_e4m3
    wq8 = np.ascontiguousarray(
        np.clip(
            norm_g[:, None] * np.asarray(Wq, np.float32) * scale * QSC, -240, 240
        )
    ).astype(f8)
    wkv = np.ascontiguousarray(cnorm_g[:, None] * np.asarray(Wkv, np.float32)).astype(bf)
    wo = np.ascontiguousarray(np.asarray(Wo, np.float32)).astype(bf)
    wff1 = np.ascontiguousarray(norm_g[:, None] * np.asarray(Wff1, np.float32)).astype(bf)
    wff2 = np.ascontiguousarray(np.asarray(Wff2, np.float32)).astype(bf)
    x_bias = bool(np.any(norm_b != 0.0))
    c_bias = bool(np.any(cnorm_b != 0.0))
    in_maps = []
    for c in range(N_CORES):
        b = c // (N_CORES // B)
        r0 = (c % (N_CORES // B)) * R
        m = {
            "x": np.ascontiguousarray(x[b, r0 : r0 + R, :]),
            "ctx": np.ascontiguousarray(context[b]),
            "wq8": wq8,
            "wkv": wkv,
            "wo": wo,
            "wff1": wff1,
            "wff2": wff2,
        }
        if x_bias:
            m["xb"] = norm_b.reshape(1, DIM).copy()
        if c_bias:
            m["cb"] = cnorm_b.reshape(1, DIM).copy()
        in_maps.append(m)
    return in_maps, x_bias, c_bias


def gather_output(results):
    out = np.empty((B, N, DIM), np.float32)
    for c in range(N_CORES):
        b = c // (N_CORES // B)
        r0 = (c % (N_CORES // B)) * R
        out[b, r0 : r0 + R, :] = results[c]["out"]
    return out


def kernel(**inputs):
    from concourse.bass_utils import run_bass_kernel_spmd

    in_maps, x_bias, c_bias = make_in_maps(**inputs)
    nc = _get_nc(x_bias, c_bias)
    res = run_bass_kernel_spmd(nc, in_maps, list(range(N_CORES)))
    return gather_output(res.results)
